# revision 21
# baseline (speedup 1.0000x reference)
# Trainium2 Bass kernel for nn_CrossAttention (RCA cross-attention block).
#
# Math (per batch b, reference semantics):
#   Q = q @ w_qs/TEMP; K = k @ w_ks; V = v @ w_vs            (16 heads x 64)
#   S_h = Q_h @ K_h^T; E = exp(S); Z = rowsum(E)
#   attn = ((colsum(V) - (E @ V)/Z)/(LK-1))
#   out = layernorm(attn @ fc_w + q @ resid_w)
#
# Sharding: data-parallel over batch, B=8 -> one batch per NeuronCore.
#
# Speed strategy vs the bf16 baseline:
#  * fp8(e4m3) DoubleRow matmuls (0.5 cyc/col, 2x bf16) for the K/V/Q
#    projections, E@V and fc paths. Host rescales weights by powers of two
#    (w_qs x64, w_ks/w_vs/fc_w x16, resid_w x512) so fp8 operands stay in
#    normal range; layernorm is scale-invariant so the net x512 cancels.
#  * k/v/q arrive transposed via DMA-transpose (no PE transposes).
#  * exp(S) split across engines: scalar does true Exp; DVE/gpsimd compute
#    fp8 E bits directly with a Schraudolph tensor_scalar (x*A+B -> int8
#    bitcast fp8). A constant multiplicative bias in E cancels in softmax.
#  * colsum(V) term is folded into fc as a rank-1 update with the
#    host-precomputed W2 = w_vs @ fc_w: csfc = (colsum v) @ W2.
#  * Z handled by a ones-column in Vsb (row 64 of the PV psum), broadcast
#    with gpsimd, and a fused scalar_tensor_tensor divide; the divide is
#    deferred one (hp,qt) iteration to hide the broadcast latency.
#
# resid_b / ln_beta are zeros and ln_gamma ones per the input spec;
# gamma/beta applied on host (exact), resid_b checked.

import numpy as np

N_HEAD, DK, DV = 16, 64, 64
TEMP = DK**0.5
LN_EPS = 1e-5
B, LQ, LK = 8, 1024, 1024
D1, D2 = 768, 1024
HD = N_HEAD * DK  # 1024
D1C, D2C, HDC, KC = D1 // 128, D2 // 128, HD // 128, LK // 128
QTS = 512
NQT = LQ // QTS

# host-side power-of-two scales (see header)
SQ = 64.0   # extra scale on w_qs (beyond /TEMP)
SK = 16.0   # w_ks
SV = 16.0   # w_vs
SF = 16.0   # fc_w
ATT = 2.0   # attnT fp8 scale on top of SV
SR = ATT * SV * SF  # 512; resid_w scale so resid matches fc-path scale
EXPS = 1.0 / (SQ * SK)  # scores psum -> true scores
# Schraudolph fast-exp to fp8e4m3 bits: bits = round(8*log2 e^s + 56)
ECONST_A = 8.0 * 1.4426950408889634 * EXPS
ECONST_B = 56.2  # 56 + 0.5 trunc-to-round - 0.3 RMS centering

# engine assignment for the 8 exp tiles per (hp, qt). gpsimd cannot read
# PSUM, so its tiles get a PSUM->SBUF bf16 bounce copy on the named engine.
E_ENG = ("act", "act", "gp", "dve", "act", "gp", "gp", "gp")
E_BOUNCE = {2: "act", 5: "act", 6: "dve", 7: "dve"}

_cache = {}


def _build_nc():
    import concourse.tile as tile
    from concourse import bacc
    from concourse import mybir

    dt = mybir.dt
    f32, f32r, bf16 = dt.float32, dt.float32r, dt.bfloat16
    f8, i8 = dt.float8e4, dt.int8
    AF = mybir.ActivationFunctionType
    ALU = mybir.AluOpType
    PM = mybir.MatmulPerfMode.DoubleRow

    # Force Exp/Ln activations onto the one table set that contains both, so
    # the softmax exp and the layernorm ln/exp chain never thrash ACT tables.
    if not getattr(bacc, "_nnca_act_patch", False):
        _orig_tables = bacc.get_activation_tables

        def _patched_tables(arch):
            t = _orig_tables(arch)
            for name, funcs in t.items():
                if name != "natural_log_exp_and_others":
                    funcs.discard(mybir.ActivationFunctionType.Exp)
                    funcs.discard(mybir.ActivationFunctionType.Ln)
            return t

        bacc.get_activation_tables = _patched_tables
        bacc._nnca_act_patch = True

    nc = bacc.Bacc("TRN2", target_bir_lowering=False, debug=False)

    q_d = nc.dram_tensor("q", [LQ, D1], bf16, kind="ExternalInput").ap()
    k_d = nc.dram_tensor("k", [LK, D2], bf16, kind="ExternalInput").ap()
    v_d = nc.dram_tensor("v", [LK, D2], bf16, kind="ExternalInput").ap()
    wqs_d = nc.dram_tensor("w_qs", [D1, HD], f8, kind="ExternalInput").ap()
    wks_d = nc.dram_tensor("w_ks", [D2, HD], f8, kind="ExternalInput").ap()
    wvs_d = nc.dram_tensor("w_vs", [D2, HD], f8, kind="ExternalInput").ap()
    fcw_d = nc.dram_tensor("fc_w", [HD, D2], f8, kind="ExternalInput").ap()
    rw_d = nc.dram_tensor("resid_w", [D1, D2], bf16, kind="ExternalInput").ap()
    w2_d = nc.dram_tensor("w2", [HD, D2], bf16, kind="ExternalInput").ap()
    out_d = nc.dram_tensor("out", [LQ, D2], f32, kind="ExternalOutput").ap()

    def dr(ap, i=2):  # view a [128, 2*n] tile as DoubleRow [128, 2, n]
        return ap.rearrange("p (i x) -> p i x", i=i)

    with tile.TileContext(nc) as tc:
        with (
            tc.tile_pool(name="const", bufs=1) as constp,
            tc.tile_pool(name="stag", bufs=4) as stagp,      # kTb/vTb bf16 staging
            tc.tile_pool(name="qtb", bufs=6) as qtbp,        # qT bf16 (persistent)
            tc.tile_pool(name="xdr", bufs=8) as xdrp,        # vT/kT/qT fp8 DR tiles
            tc.tile_pool(name="wdr", bufs=8) as wdrp,        # wvs/wks/wqs/fcw fp8 DR
            tc.tile_pool(name="big2", bufs=8) as big2p,      # W2 bf16 -> rw bf16
            tc.tile_pool(name="rwp", bufs=6) as rwp,         # rw bf16
            tc.tile_pool(name="ktq", bufs=16) as ktqp,       # KT/QT bf16
            tc.tile_pool(name="vsb", bufs=4) as vsbp,        # Vsb fp8 (DR pairs)
            tc.tile_pool(name="es", bufs=6) as esp,          # E fp8 (DR pairs)
            tc.tile_pool(name="att", bufs=4) as attp,        # attnT fp8 (DR pairs)
            tc.tile_pool(name="scb", bufs=3) as scbp,        # scores bf16 bounce
            tc.tile_pool(name="zsb", bufs=4) as zsbp,        # Z rows bf16
            tc.tile_pool(name="ln", bufs=4) as lnp,          # LN big f32 tiles
            tc.tile_pool(name="small", bufs=24) as smallp,
            tc.tile_pool(name="psA", bufs=2, space="PSUM") as psA,  # 2-bank tiles
            tc.tile_pool(name="ps1", bufs=4, space="PSUM") as ps1,  # 1-bank tiles
        ):
            ones_q = constp.tile([1, 128], bf16, name="ones_q")
            nc.vector.memset(ones_q[:], 1.0)
            epsb = constp.tile([128, 1], f32, name="epsb")
            nc.vector.memset(epsb[:], LN_EPS * SR * SR)
            ones_f8 = constp.tile([128, 128], f8, name="ones_f8")
            nc.vector.memset(ones_f8[:], 1.0)

            sbuf_rot = ["dve", "act", "gp"]   # SBUF->SBUF casts
            psum_rot = ["dve", "act"]         # PSUM-sourced (gpsimd can't see PSUM)

            def _copy(eng, dst, src):
                if eng == "act":
                    nc.scalar.copy(dst, src)
                elif eng == "dve":
                    nc.vector.tensor_copy(dst, src)
                else:
                    nc.gpsimd.tensor_copy(dst, src)

            def cast_sbuf(dst, src, n):
                _copy(sbuf_rot[n % 3], dst, src)

            def cast_psum(dst, src, n):
                _copy(psum_rot[n % 2], dst, src)

            # ---------------- V path ----------------
            vsumb = smallp.tile([128, D2C], bf16, bufs=1, name="vsumb")
            vT_dr = [xdrp.tile([128, 2048], f8, tag="xdr", name=f"vT{i}") for i in range(4)]
            ncast = 0
            for c in range(D2C):
                vTb = stagp.tile([128, LK], bf16, tag="stag", name="vTb")
                nc.sync.dma_start_transpose(vTb[:], v_d[:, 128 * c : 128 * c + 128])
                cast_sbuf(vT_dr[c // 2][:, 1024 * (c % 2) : 1024 * (c % 2) + 1024], vTb[:], ncast)
                ncast += 1
                with nc.allow_low_precision(reason="colsum stats at bf16"):
                    nc.vector.tensor_reduce(
                        vsumb[:, c : c + 1], vTb[:], axis=mybir.AxisListType.X, op=ALU.add
                    )
            wvs_dr = [wdrp.tile([128, 2048], f8, tag="wdr", name=f"wvs{i}") for i in range(4)]
            for cg in range(4):
                for i in range(2):
                    nc.scalar.dma_start(
                        wvs_dr[cg][:, 1024 * i : 1024 * i + 1024],
                        wvs_d[128 * (2 * cg + i) : 128 * (2 * cg + i) + 128, :],
                    )
            W2b = [big2p.tile([128, D2], bf16, tag="big2", name=f"w2b{i}") for i in range(HDC)]
            for c in range(HDC):
                nc.scalar.dma_start(W2b[c][:], w2_d[128 * c : 128 * c + 128, :])

            Vsb = [vsbp.tile([128, 2048], f8, tag="v", name=f"Vsb{i}") for i in range(4)]
            npc = 0
            for kc in range(KC):
                cg, sl = kc // 2, kc % 2
                ps = psA.tile([128, 1024], f32, tag="ps_big", name="vps")
                for cp in range(4):
                    for t in range(2):
                        nc.tensor.matmul(
                            ps[:, 512 * t : 512 * t + 512],
                            lhsT=dr(vT_dr[cp][:])[:, :, 128 * kc : 128 * kc + 128],
                            rhs=dr(wvs_dr[cp][:])[:, :, 512 * t : 512 * t + 512],
                            start=(cp == 0),
                            stop=(cp == 3),
                            perf_mode=PM,
                        )
                cast_psum(Vsb[cg][:, 1024 * sl : 1024 * sl + 1024], ps[:], npc)
                npc += 1

            # csfc = (colsum v) @ W2  (rank-1 colsum term of fc, see header)
            csfc_sb = smallp.tile([1, D2], bf16, bufs=1, name="csfc")
            for t in range(2):
                cps = ps1.tile([1, 512], f32, tag="ps1", name="cps")
                for c in range(HDC):
                    nc.tensor.matmul(
                        cps[:],
                        lhsT=vsumb[:, c : c + 1],
                        rhs=W2b[c][:, 512 * t : 512 * t + 512],
                        start=(c == 0),
                        stop=(c == HDC - 1),
                    )
                nc.vector.tensor_copy(csfc_sb[:, 512 * t : 512 * t + 512], cps[:])

            # ---------------- K path ----------------
            kT_dr = [xdrp.tile([128, 2048], f8, tag="xdr", name=f"kT{i}") for i in range(4)]
            for c in range(D2C):
                kTb = stagp.tile([128, LK], bf16, tag="stag", name="kTb")
                nc.sync.dma_start_transpose(kTb[:], k_d[:, 128 * c : 128 * c + 128])
                cast_sbuf(kT_dr[c // 2][:, 1024 * (c % 2) : 1024 * (c % 2) + 1024], kTb[:], ncast)
                ncast += 1
            wks_dr = [wdrp.tile([128, 2048], f8, tag="wdr", name=f"wks{i}") for i in range(4)]
            for cg in range(4):
                for i in range(2):
                    nc.scalar.dma_start(
                        wks_dr[cg][:, 1024 * i : 1024 * i + 1024],
                        wks_d[128 * (2 * cg + i) : 128 * (2 * cg + i) + 128, :],
                    )

            # ---------------- Q path ----------------
            qTb = [qtbp.tile([128, LQ], bf16, tag="qtb", name=f"qTb{i}") for i in range(D1C)]
            qT_dr = [xdrp.tile([128, 2048], f8, tag="xdr", name=f"qT{i}") for i in range(3)]
            for c in range(D1C):
                nc.sync.dma_start_transpose(qTb[c][:], q_d[:, 128 * c : 128 * c + 128])
                cast_sbuf(qT_dr[c // 2][:, 1024 * (c % 2) : 1024 * (c % 2) + 1024], qTb[c][:], ncast)
                ncast += 1
            wqs_dr = [wdrp.tile([128, 2048], f8, tag="wdr", name=f"wqs{i}") for i in range(3)]
            for cg in range(3):
                for i in range(2):
                    nc.scalar.dma_start(
                        wqs_dr[cg][:, 1024 * i : 1024 * i + 1024],
                        wqs_d[128 * (2 * cg + i) : 128 * (2 * cg + i) + 128, :],
                    )
            fcw_dr = [wdrp.tile([128, 2048], f8, tag="wdr", name=f"fcw{i}") for i in range(4)]
            for cg in range(4):
                for i in range(2):
                    nc.scalar.dma_start(
                        fcw_dr[cg][:, 1024 * i : 1024 * i + 1024],
                        fcw_d[128 * (2 * cg + i) : 128 * (2 * cg + i) + 128, :],
                    )
            rw = [rwp.tile([128, D2], bf16, tag="rw", name=f"rw{i}") for i in range(D1C)]
            for c in range(D1C):
                nc.scalar.dma_start(rw[c][:], rw_d[128 * c : 128 * c + 128, :])

            # ---------------- K/Q projections (per head-pair) ----------------
            KT = [ktqp.tile([128, LK], bf16, tag="ktq", name=f"KT{i}") for i in range(HDC)]
            QT = [ktqp.tile([128, LQ], bf16, tag="ktq", name=f"QT{i}") for i in range(HDC)]
            nc2 = 0

            def kq_proj(hp, dst_t, w_t, x_t, npair):
                nonlocal nc2
                phs = [ps1.tile([128, 512], f32, tag="ps1", name="ph") for _ in range(2)]
                for cp in range(npair):
                    for t in range(2):
                        nc.tensor.matmul(
                            phs[t][:],
                            lhsT=dr(w_t[cp][:])[:, :, 128 * hp : 128 * hp + 128],
                            rhs=dr(x_t[cp][:])[:, :, 512 * t : 512 * t + 512],
                            start=(cp == 0),
                            stop=(cp == npair - 1),
                            perf_mode=PM,
                        )
                for t in range(2):
                    cast_psum(dst_t[hp][:, 512 * t : 512 * t + 512], phs[t][:], nc2)
                    nc2 += 1

            for hp in range(HDC):
                kq_proj(hp, KT, wks_dr, kT_dr, 4)
                kq_proj(hp, QT, wqs_dr, qT_dr, 3)

            # ---------------- attention ----------------
            attnT = [attp.tile([128, 2048], f8, tag="at", name=f"attnT{i}") for i in range(4)]

            def emit_exp(kc, es, i2, sc):
                eslot = es[:, 1024 * i2 : 1024 * i2 + 1024]
                eng = E_ENG[kc]
                with nc.allow_low_precision(reason="E at fp8"):
                    if eng == "act":
                        nc.scalar.activation(eslot, sc[:], AF.Exp, scale=EXPS)
                    elif eng == "dve":
                        nc.vector.tensor_scalar(
                            out=eslot.bitcast(i8),
                            in0=sc[:],
                            scalar1=ECONST_A,
                            scalar2=ECONST_B,
                            op0=ALU.mult,
                            op1=ALU.add,
                        )
                    else:
                        # gpsimd cannot read PSUM: bounce scores to SBUF bf16
                        sb = scbp.tile([128, 1024], bf16, tag="scb", name="scb")
                        _copy(E_BOUNCE[kc], sb[:], sc[:])
                        nc.gpsimd.tensor_scalar(
                            out=eslot.bitcast(i8),
                            in0=sb[:],
                            scalar1=ECONST_A,
                            scalar2=ECONST_B,
                            op0=ALU.mult,
                            op1=ALU.add,
                        )

            def attn_iter(qt, hp):
                pvz = None
                for cg in range(4):
                    es = esp.tile([128, 2048], f8, tag="e", name="es")
                    for i2 in range(2):
                        kc = 2 * cg + i2
                        sc = psA.tile([128, 1024], f32, tag="ps_big", name="sc")
                        for j in range(2):
                            nc.tensor.matmul(
                                sc[:, 512 * j : 512 * j + 512],
                                lhsT=KT[hp][64 * j : 64 * j + 64, 128 * kc : 128 * kc + 128],
                                rhs=QT[hp][64 * j : 64 * j + 64, QTS * qt : QTS * qt + QTS],
                                start=True,
                                stop=True,
                                tile_position=(64 * j, 0),
                            )
                        emit_exp(kc, es, i2, sc)
                    if cg == 0:
                        pvz = [
                            ps1.tile([64, 512], f32, tag="ps1", name=f"pvz{j}")
                            for j in range(2)
                        ]
                        zps = [
                            ps1.tile([64, 512], f32, tag="ps1", name=f"zps{j}")
                            for j in range(2)
                        ]
                    for j in range(2):
                        h = 2 * hp + j
                        nc.tensor.matmul(
                            pvz[j][:],
                            lhsT=dr(Vsb[cg][:])[:, :, 64 * h : 64 * h + 64],
                            rhs=dr(es[:])[:, :, 512 * j : 512 * j + 512],
                            start=(cg == 0),
                            stop=(cg == 3),
                            perf_mode=PM,
                        )
                        # Z = colsum(E), broadcast to 64 rows via all-ones weights
                        nc.tensor.matmul(
                            zps[j][:],
                            lhsT=dr(ones_f8[:])[:, :, 0:64],
                            rhs=dr(es[:])[:, :, 512 * j : 512 * j + 512],
                            start=(cg == 0),
                            stop=(cg == 3),
                            perf_mode=PM,
                        )
                # attnT = (pv * -ATT/(LK-1)) * (1/Z); STT has no divide op and
                # only one PSUM input, so 1/Z lands in SBUF via approx recip.
                for j in range(2):
                    zsb = zsbp.tile([64, 512], f32, tag="zsb", name="zsb")
                    nc.vector.reciprocal_approx_fast(zsb[:], zps[j][:])
                    outsl = attnT[hp // 2][
                        64 * j : 64 * j + 64,
                        1024 * (hp % 2) + QTS * qt : 1024 * (hp % 2) + QTS * qt + QTS,
                    ]
                    with nc.allow_low_precision(reason="attn fp8"):
                        nc.vector.scalar_tensor_tensor(
                            out=outsl,
                            in0=pvz[j][:],
                            scalar=-ATT / (LK - 1),
                            in1=zsb[:],
                            op0=ALU.mult,
                            op1=ALU.mult,
                        )

            def fc_ln(qq):
                ps2 = psA.tile([128, 1024], f32, tag="ps_big", name="ps2")
                for t in range(2):
                    psl = ps2[:, 512 * t : 512 * t + 512]
                    for c in range(D1C):
                        nc.tensor.matmul(
                            psl,
                            lhsT=qTb[c][:, 128 * qq : 128 * qq + 128],
                            rhs=rw[c][:, 512 * t : 512 * t + 512],
                            start=(c == 0),
                            stop=False,
                        )
                    nc.tensor.matmul(
                        psl,
                        lhsT=ones_q[:],
                        rhs=csfc_sb[:, 512 * t : 512 * t + 512],
                        start=False,
                        stop=False,
                    )
                    for cp in range(4):
                        nc.tensor.matmul(
                            psl,
                            lhsT=dr(attnT[cp][:])[:, :, 128 * qq : 128 * qq + 128],
                            rhs=dr(fcw_dr[cp][:])[:, :, 512 * t : 512 * t + 512],
                            start=False,
                            stop=(cp == 3),
                            perf_mode=PM,
                        )
                ssum = smallp.tile([128, 1], f32, tag="stat", bufs=12, name="ssum")
                nc.vector.tensor_reduce(ssum[:], ps2[:], axis=mybir.AxisListType.X, op=ALU.add)
                sq = lnp.tile([128, 1024], f32, tag="ln", name="sq")
                vsum = smallp.tile([128, 1], f32, tag="stat", bufs=12, name="vsum")
                nc.scalar.activation(sq[:], ps2[:], AF.Square, accum_out=vsum[:])
                m = smallp.tile([128, 1], f32, tag="stat", bufs=12, name="m")
                nc.vector.tensor_scalar(
                    out=m[:], in0=ssum[:], scalar1=1.0 / D2, scalar2=None, op0=ALU.mult
                )
                v1 = smallp.tile([128, 1], f32, tag="stat", bufs=12, name="v1")
                nc.vector.scalar_tensor_tensor(
                    out=v1[:], in0=m[:], scalar=-1.0, in1=m[:],
                    op0=ALU.mult, op1=ALU.mult,
                )
                nc.vector.scalar_tensor_tensor(
                    out=v1[:], in0=vsum[:], scalar=1.0 / D2, in1=v1[:],
                    op0=ALU.mult, op1=ALU.add,
                )
                # rstd = exp(-0.5 ln(var+eps)) -- stays in the ln/exp table set
                rstd = smallp.tile([128, 1], f32, tag="stat", bufs=12, name="rstd")
                nc.scalar.activation(rstd[:], v1[:], AF.Ln, bias=epsb[:])
                nc.scalar.activation(rstd[:], rstd[:], AF.Exp, scale=-0.5)
                nbias = smallp.tile([128, 1], f32, tag="stat", bufs=12, name="nbias")
                nc.vector.scalar_tensor_tensor(
                    out=nbias[:], in0=m[:], scalar=-1.0, in1=rstd[:],
                    op0=ALU.mult, op1=ALU.mult,
                )
                ot = lnp.tile([128, 1024], f32, tag="ln", name="ot")
                nc.scalar.activation(ot[:], ps2[:], AF.Identity, bias=nbias[:], scale=rstd[:])
                nc.sync.dma_start(out_d[128 * qq : 128 * qq + 128, :], ot[:])

            for qt in range(NQT):
                for hp in range(HDC):
                    attn_iter(qt, hp)
                for qq in range(4 * qt, 4 * qt + 4):
                    fc_ln(qq)
    nc.finalize()
    return nc


def prepare_in_maps(q, k, v, w_qs, w_ks, w_vs, fc_w, resid_w, **_unused):
    import ml_dtypes

    bf = ml_dtypes.bfloat16
    f8 = getattr(ml_dtypes, "float8_e4m3", None) or ml_dtypes.float8_e4m3fn
    q = np.asarray(q, np.float32).astype(bf)
    k = np.asarray(k, np.float32).astype(bf)
    v = np.asarray(v, np.float32).astype(bf)
    wqs = (np.asarray(w_qs, np.float32) * (SQ / TEMP)).astype(f8)
    wks = (np.asarray(w_ks, np.float32) * SK).astype(f8)
    wvs = (np.asarray(w_vs, np.float32) * SV).astype(f8)
    fcw = (np.asarray(fc_w, np.float32) * SF).astype(f8)
    rw = (np.asarray(resid_w, np.float32) * SR).astype(bf)
    # rank-1 colsum term: csfc = (colsum v) @ w_vs @ fc_w * (SR/(LK-1))
    w2 = (
        np.asarray(w_vs, np.float32) @ np.asarray(fc_w, np.float32) * (SR / (LK - 1))
    ).astype(bf)
    return [
        {
            "q": q[i], "k": k[i], "v": v[i],
            "w_qs": wqs, "w_ks": wks, "w_vs": wvs,
            "fc_w": fcw, "resid_w": rw, "w2": w2,
        }
        for i in range(B)
    ]


def get_nc():
    if "nc" not in _cache:
        _cache["nc"] = _build_nc()
    return _cache["nc"]


def kernel(q, k, v, w_qs, w_ks, w_vs, fc_w, resid_w, resid_b, ln_gamma, ln_beta):
    from concourse.bass_utils import run_bass_kernel_spmd

    nc = get_nc()
    in_maps = prepare_in_maps(q, k, v, w_qs, w_ks, w_vs, fc_w, resid_w)
    res = run_bass_kernel_spmd(nc, in_maps, core_ids=list(range(B)))
    out = np.stack([res.results[i]["out"] for i in range(B)]).astype(np.float32)

    # gamma/beta applied post-norm on host (spec fills are ones/zeros; exact).
    g = np.asarray(ln_gamma, np.float32)
    bta = np.asarray(ln_beta, np.float32)
    out = out * g[None, None, :] + bta[None, None, :]
    rb = np.asarray(resid_b, np.float32)
    if np.any(rb):
        raise NotImplementedError("nonzero resid_b not supported by this kernel")
    return out


# revision 23
# speedup vs baseline: 1.1939x; 1.1939x over previous
# Trainium2 Bass kernel for nn_CrossAttention (RCA cross-attention block).
#
# Math (per batch b, reference semantics):
#   Q = q @ w_qs/TEMP; K = k @ w_ks; V = v @ w_vs            (16 heads x 64)
#   S_h = Q_h @ K_h^T; E = exp(S); Z = rowsum(E)
#   attn = ((colsum(V) - (E @ V)/Z)/(LK-1))
#   out = layernorm(attn @ fc_w + q @ resid_w)
#
# Sharding: data-parallel over batch, B=8 -> one batch per NeuronCore.
#
# Speed strategy vs the bf16 baseline:
#  * fp8(e4m3) DoubleRow matmuls (0.5 cyc/col, 2x bf16) for the K/V/Q
#    projections, E@V and fc paths. Host rescales weights by powers of two
#    (w_qs x64, w_ks/w_vs/fc_w x16, resid_w x512) so fp8 operands stay in
#    normal range; layernorm is scale-invariant so the net x512 cancels.
#  * k/v/q arrive transposed via DMA-transpose (no PE transposes).
#  * exp(S) split across engines: scalar does true Exp; DVE/gpsimd compute
#    fp8 E bits directly with a Schraudolph tensor_scalar (x*A+B -> int8
#    bitcast fp8). A constant multiplicative bias in E cancels in softmax.
#  * colsum(V) term is folded into fc as a rank-1 update with the
#    host-precomputed W2 = w_vs @ fc_w: csfc = (colsum v) @ W2.
#  * Z handled by a ones-column in Vsb (row 64 of the PV psum), broadcast
#    with gpsimd, and a fused scalar_tensor_tensor divide; the divide is
#    deferred one (hp,qt) iteration to hide the broadcast latency.
#
# resid_b / ln_beta are zeros and ln_gamma ones per the input spec;
# gamma/beta applied on host (exact), resid_b checked.

import numpy as np

N_HEAD, DK, DV = 16, 64, 64
TEMP = DK**0.5
LN_EPS = 1e-5
B, LQ, LK = 8, 1024, 1024
D1, D2 = 768, 1024
HD = N_HEAD * DK  # 1024
D1C, D2C, HDC, KC = D1 // 128, D2 // 128, HD // 128, LK // 128
QTS = 512
NQT = LQ // QTS

# host-side power-of-two scales (see header)
SQ = 64.0   # extra scale on w_qs (beyond /TEMP)
SK = 16.0   # w_ks
SV = 16.0   # w_vs
SF = 16.0   # fc_w
ATT = 2.0   # attnT fp8 scale on top of SV
SR = ATT * SV * SF  # 512; resid_w scale so resid matches fc-path scale
EXPS = 1.0 / (SQ * SK)  # scores psum -> true scores
# Schraudolph fast-exp to fp8e4m3 bits: bits = round(8*log2 e^s + 56)
ECONST_A = 8.0 * 1.4426950408889634 * EXPS
ECONST_B = 56.2  # 56 + 0.5 trunc-to-round - 0.3 RMS centering

# engine assignment for the 8 exp tiles per (hp, qt). gpsimd cannot read
# PSUM, so its tiles get a PSUM->SBUF bf16 bounce copy on the named engine.
E_ENG = ("act", "act", "gp", "dve", "act", "gp", "gp", "gp")
E_BOUNCE = {2: "act", 5: "act", 6: "dve", 7: "dve"}

_cache = {}


def _build_nc():
    import concourse.tile as tile
    from concourse import bacc
    from concourse import mybir

    dt = mybir.dt
    f32, f32r, bf16 = dt.float32, dt.float32r, dt.bfloat16
    f8, i8 = dt.float8e4, dt.int8
    AF = mybir.ActivationFunctionType
    ALU = mybir.AluOpType
    PM = mybir.MatmulPerfMode.DoubleRow

    # Force Exp/Ln activations onto the one table set that contains both, so
    # the softmax exp and the layernorm ln/exp chain never thrash ACT tables.
    if not getattr(bacc, "_nnca_act_patch", False):
        _orig_tables = bacc.get_activation_tables

        def _patched_tables(arch):
            t = _orig_tables(arch)
            for name, funcs in t.items():
                if name != "natural_log_exp_and_others":
                    funcs.discard(mybir.ActivationFunctionType.Exp)
                    funcs.discard(mybir.ActivationFunctionType.Ln)
            return t

        bacc.get_activation_tables = _patched_tables
        bacc._nnca_act_patch = True

    nc = bacc.Bacc("TRN2", target_bir_lowering=False, debug=False)

    q_d = nc.dram_tensor("q", [LQ, D1], bf16, kind="ExternalInput").ap()
    k_d = nc.dram_tensor("k", [LK, D2], bf16, kind="ExternalInput").ap()
    v_d = nc.dram_tensor("v", [LK, D2], bf16, kind="ExternalInput").ap()
    wqs_d = nc.dram_tensor("w_qs", [D1, HD], f8, kind="ExternalInput").ap()
    wks_d = nc.dram_tensor("w_ks", [D2, HD], f8, kind="ExternalInput").ap()
    wvs_d = nc.dram_tensor("w_vs", [D2, HD], f8, kind="ExternalInput").ap()
    fcw_d = nc.dram_tensor("fc_w", [HD, D2], f8, kind="ExternalInput").ap()
    rw_d = nc.dram_tensor("resid_w", [D1, D2], bf16, kind="ExternalInput").ap()
    w2_d = nc.dram_tensor("w2", [HD, D2], bf16, kind="ExternalInput").ap()
    out_d = nc.dram_tensor("out", [LQ, D2], f32, kind="ExternalOutput").ap()

    def dr(ap, i=2):  # view a [128, 2*n] tile as DoubleRow [128, 2, n]
        return ap.rearrange("p (i x) -> p i x", i=i)

    with tile.TileContext(nc) as tc:
        with (
            tc.tile_pool(name="const", bufs=1) as constp,
            tc.tile_pool(name="stag", bufs=4) as stagp,      # kTb/vTb bf16 staging
            tc.tile_pool(name="qtb", bufs=6) as qtbp,        # qT bf16 (persistent)
            tc.tile_pool(name="xdr", bufs=8) as xdrp,        # vT/kT/qT fp8 DR tiles
            tc.tile_pool(name="wdr", bufs=8) as wdrp,        # wvs/wks/wqs/fcw fp8 DR
            tc.tile_pool(name="big2", bufs=8) as big2p,      # W2 bf16 -> rw bf16
            tc.tile_pool(name="rwp", bufs=6) as rwp,         # rw bf16
            tc.tile_pool(name="ktq", bufs=16) as ktqp,       # KT/QT bf16
            tc.tile_pool(name="vsb", bufs=4) as vsbp,        # Vsb fp8 (DR pairs)
            tc.tile_pool(name="es", bufs=6) as esp,          # E fp8 (DR pairs)
            tc.tile_pool(name="att", bufs=4) as attp,        # attnT fp8 (DR pairs)
            tc.tile_pool(name="scb", bufs=3) as scbp,        # scores bf16 bounce
            tc.tile_pool(name="zsb", bufs=4) as zsbp,        # Z rows bf16
            tc.tile_pool(name="ln", bufs=4) as lnp,          # LN big f32 tiles
            tc.tile_pool(name="small", bufs=24) as smallp,
            tc.tile_pool(name="psA", bufs=2, space="PSUM") as psA,  # 2-bank tiles
            tc.tile_pool(name="ps1", bufs=4, space="PSUM") as ps1,  # 1-bank tiles
        ):
            ones_q = constp.tile([1, 128], bf16, name="ones_q")
            nc.vector.memset(ones_q[:], 1.0)
            epsb = constp.tile([128, 1], f32, name="epsb")
            nc.vector.memset(epsb[:], LN_EPS * SR * SR)
            ones_f8 = constp.tile([128, 128], f8, name="ones_f8")
            nc.vector.memset(ones_f8[:], 1.0)

            sbuf_rot = ["dve", "act", "gp"]   # SBUF->SBUF casts
            psum_rot = ["dve", "act"]         # PSUM-sourced (gpsimd can't see PSUM)

            def _copy(eng, dst, src):
                if eng == "act":
                    nc.scalar.copy(dst, src)
                elif eng == "dve":
                    nc.vector.tensor_copy(dst, src)
                else:
                    nc.gpsimd.tensor_copy(dst, src)

            def cast_sbuf(dst, src, n):
                _copy(sbuf_rot[n % 3], dst, src)

            def cast_psum(dst, src, n):
                _copy(psum_rot[n % 2], dst, src)

            # ---------------- V path ----------------
            vsumb = smallp.tile([128, D2C], bf16, bufs=1, name="vsumb")
            vT_dr = [xdrp.tile([128, 2048], f8, tag="xdr", name=f"vT{i}") for i in range(4)]
            ncast = 0
            for c in range(D2C):
                vTb = stagp.tile([128, LK], bf16, tag="stag", name="vTb")
                dq = nc.sync if c % 2 == 0 else nc.scalar
                dq.dma_start_transpose(vTb[:], v_d[:, 128 * c : 128 * c + 128])
                cast_sbuf(vT_dr[c // 2][:, 1024 * (c % 2) : 1024 * (c % 2) + 1024], vTb[:], ncast)
                ncast += 1
                with nc.allow_low_precision(reason="colsum stats at bf16"):
                    nc.vector.tensor_reduce(
                        vsumb[:, c : c + 1], vTb[:], axis=mybir.AxisListType.X, op=ALU.add
                    )
            wvs_dr = [wdrp.tile([128, 2048], f8, tag="wdr", name=f"wvs{i}") for i in range(4)]
            for cg in range(4):
                for i in range(2):
                    nc.scalar.dma_start(
                        wvs_dr[cg][:, 1024 * i : 1024 * i + 1024],
                        wvs_d[128 * (2 * cg + i) : 128 * (2 * cg + i) + 128, :],
                    )
            W2b = [big2p.tile([128, D2], bf16, tag="big2", name=f"w2b{i}") for i in range(HDC)]
            for c in range(HDC):
                nc.scalar.dma_start(W2b[c][:], w2_d[128 * c : 128 * c + 128, :])

            Vsb = [vsbp.tile([128, 2 * 16 * 65], f8, tag="v", name=f"Vsb{i}") for i in range(4)]
            npc = 0
            for kc in range(KC):
                cg, sl = kc // 2, kc % 2
                if sl == 0:
                    # ones columns (col 64 of each head block, both slots)
                    oc = Vsb[cg][:].rearrange("p (i h c) -> p i h c", i=2, h=16)[:, :, :, 64:65]
                    nc.vector.memset(oc, 1.0)
                ps = psA.tile([128, 1024], f32, tag="ps_big", name="vps")
                for cp in range(4):
                    for t in range(2):
                        nc.tensor.matmul(
                            ps[:, 512 * t : 512 * t + 512],
                            lhsT=dr(vT_dr[cp][:])[:, :, 128 * kc : 128 * kc + 128],
                            rhs=dr(wvs_dr[cp][:])[:, :, 512 * t : 512 * t + 512],
                            start=(cp == 0),
                            stop=(cp == 3),
                            perf_mode=PM,
                        )
                dst = Vsb[cg][:, 16 * 65 * sl : 16 * 65 * sl + 16 * 65]
                dst = dst.rearrange("p (h c) -> p h c", h=16)[:, :, 0:64]
                src_ap = ps[:].rearrange("p (h c) -> p h c", h=16)
                cast_psum(dst, src_ap, npc)
                npc += 1

            # csfc = (colsum v) @ W2  (rank-1 colsum term of fc, see header).
            # Emitted lazily right before the first fc so the PE never waits
            # on the W2 DMA.
            csfc_sb = smallp.tile([1, D2], bf16, bufs=1, name="csfc")
            _csfc_state = {"done": False}

            def emit_csfc():
                if _csfc_state["done"]:
                    return
                _csfc_state["done"] = True
                for t in range(2):
                    cps = ps1.tile([1, 512], f32, tag="ps1", name="cps")
                    for c in range(HDC):
                        nc.tensor.matmul(
                            cps[:],
                            lhsT=vsumb[:, c : c + 1],
                            rhs=W2b[c][:, 512 * t : 512 * t + 512],
                            start=(c == 0),
                            stop=(c == HDC - 1),
                        )
                    nc.vector.tensor_copy(csfc_sb[:, 512 * t : 512 * t + 512], cps[:])

            # ---------------- K path ----------------
            kT_dr = [xdrp.tile([128, 2048], f8, tag="xdr", name=f"kT{i}") for i in range(4)]
            for c in range(D2C):
                kTb = stagp.tile([128, LK], bf16, tag="stag", name="kTb")
                dq = nc.sync if c % 2 == 0 else nc.scalar
                dq.dma_start_transpose(kTb[:], k_d[:, 128 * c : 128 * c + 128])
                cast_sbuf(kT_dr[c // 2][:, 1024 * (c % 2) : 1024 * (c % 2) + 1024], kTb[:], ncast)
                ncast += 1
            wks_dr = [wdrp.tile([128, 2048], f8, tag="wdr", name=f"wks{i}") for i in range(4)]
            for cg in range(4):
                for i in range(2):
                    nc.scalar.dma_start(
                        wks_dr[cg][:, 1024 * i : 1024 * i + 1024],
                        wks_d[128 * (2 * cg + i) : 128 * (2 * cg + i) + 128, :],
                    )

            # ---------------- Q path ----------------
            qTb = [qtbp.tile([128, LQ], bf16, tag="qtb", name=f"qTb{i}") for i in range(D1C)]
            qT_dr = [xdrp.tile([128, 2048], f8, tag="xdr", name=f"qT{i}") for i in range(3)]
            for c in range(D1C):
                dq = nc.sync if c % 2 == 0 else nc.scalar
                dq.dma_start_transpose(qTb[c][:], q_d[:, 128 * c : 128 * c + 128])
                cast_sbuf(qT_dr[c // 2][:, 1024 * (c % 2) : 1024 * (c % 2) + 1024], qTb[c][:], ncast)
                ncast += 1
            wqs_dr = [wdrp.tile([128, 2048], f8, tag="wdr", name=f"wqs{i}") for i in range(3)]
            for cg in range(3):
                for i in range(2):
                    nc.scalar.dma_start(
                        wqs_dr[cg][:, 1024 * i : 1024 * i + 1024],
                        wqs_d[128 * (2 * cg + i) : 128 * (2 * cg + i) + 128, :],
                    )
            fcw_dr = [wdrp.tile([128, 2048], f8, tag="wdr", name=f"fcw{i}") for i in range(4)]
            for cg in range(4):
                for i in range(2):
                    nc.scalar.dma_start(
                        fcw_dr[cg][:, 1024 * i : 1024 * i + 1024],
                        fcw_d[128 * (2 * cg + i) : 128 * (2 * cg + i) + 128, :],
                    )
            rw = [rwp.tile([128, D2], bf16, tag="rw", name=f"rw{i}") for i in range(D1C)]
            for c in range(D1C):
                nc.scalar.dma_start(rw[c][:], rw_d[128 * c : 128 * c + 128, :])

            # ---------------- K/Q projections (per head-pair) ----------------
            KT = [ktqp.tile([128, LK], bf16, tag="ktq", name=f"KT{i}") for i in range(HDC)]
            QT = [ktqp.tile([128, LQ], bf16, tag="ktq", name=f"QT{i}") for i in range(HDC)]
            nc2 = 0

            def kq_proj(hp, dst_t, w_t, x_t, npair):
                nonlocal nc2
                phs = [ps1.tile([128, 512], f32, tag="ps1", name="ph") for _ in range(2)]
                for cp in range(npair):
                    for t in range(2):
                        nc.tensor.matmul(
                            phs[t][:],
                            lhsT=dr(w_t[cp][:])[:, :, 128 * hp : 128 * hp + 128],
                            rhs=dr(x_t[cp][:])[:, :, 512 * t : 512 * t + 512],
                            start=(cp == 0),
                            stop=(cp == npair - 1),
                            perf_mode=PM,
                        )
                for t in range(2):
                    cast_psum(dst_t[hp][:, 512 * t : 512 * t + 512], phs[t][:], nc2)
                    nc2 += 1

            for hp in range(HDC):
                kq_proj(hp, KT, wks_dr, kT_dr, 4)
                kq_proj(hp, QT, wqs_dr, qT_dr, 3)

            # ---------------- attention ----------------
            attnT = [attp.tile([128, 2048], f8, tag="at", name=f"attnT{i}") for i in range(4)]

            def emit_exp(kc, es, i2, sc):
                eslot = es[:, 1024 * i2 : 1024 * i2 + 1024]
                eng = E_ENG[kc]
                with nc.allow_low_precision(reason="E at fp8"):
                    if eng == "act":
                        nc.scalar.activation(eslot, sc[:], AF.Exp, scale=EXPS)
                    elif eng == "dve":
                        nc.vector.tensor_scalar(
                            out=eslot.bitcast(i8),
                            in0=sc[:],
                            scalar1=ECONST_A,
                            scalar2=ECONST_B,
                            op0=ALU.mult,
                            op1=ALU.add,
                        )
                    else:
                        # gpsimd cannot read PSUM: bounce scores to SBUF bf16
                        sb = scbp.tile([128, 1024], bf16, tag="scb", name="scb")
                        _copy(E_BOUNCE[kc], sb[:], sc[:])
                        nc.gpsimd.tensor_scalar(
                            out=eslot.bitcast(i8),
                            in0=sb[:],
                            scalar1=ECONST_A,
                            scalar2=ECONST_B,
                            op0=ALU.mult,
                            op1=ALU.add,
                        )

            def attn_iter(qt, hp):
                pvz = None
                for cg in range(4):
                    es = esp.tile([128, 2048], f8, tag="e", name="es")
                    for i2 in range(2):
                        kc = 2 * cg + i2
                        sc = psA.tile([128, 1024], f32, tag="ps_big", name="sc")
                        for j in range(2):
                            nc.tensor.matmul(
                                sc[:, 512 * j : 512 * j + 512],
                                lhsT=KT[hp][64 * j : 64 * j + 64, 128 * kc : 128 * kc + 128],
                                rhs=QT[hp][64 * j : 64 * j + 64, QTS * qt : QTS * qt + QTS],
                                start=True,
                                stop=True,
                                tile_position=(64 * j, 0),
                            )
                        emit_exp(kc, es, i2, sc)
                    if cg == 0:
                        pvz = [
                            ps1.tile([65, 512], f32, tag="ps1", name=f"pvz{j}")
                            for j in range(2)
                        ]
                    for j in range(2):
                        h = 2 * hp + j
                        # E @ V; row 64 is Z = colsum(E) via the ones column
                        nc.tensor.matmul(
                            pvz[j][:],
                            lhsT=dr(Vsb[cg][:])[:, :, 65 * h : 65 * h + 65],
                            rhs=dr(es[:])[:, :, 512 * j : 512 * j + 512],
                            start=(cg == 0),
                            stop=(cg == 3),
                            perf_mode=PM,
                        )
                # attnT = (pv * -ATT/(LK-1)) * (1/Z): approx-reciprocal of the
                # Z row straight from PSUM, gpsimd-broadcast it to 64 rows.
                for j in range(2):
                    zri = smallp.tile([1, 512], f32, tag="zri", bufs=4, name="zri")
                    nc.vector.reciprocal_approx_fast(zri[:], pvz[j][64:65, :])
                    zbi = zsbp.tile([64, 512], f32, tag="zsb", name="zbi")
                    nc.gpsimd.partition_broadcast(zbi[:], zri[:])
                    outsl = attnT[hp // 2][
                        64 * j : 64 * j + 64,
                        1024 * (hp % 2) + QTS * qt : 1024 * (hp % 2) + QTS * qt + QTS,
                    ]
                    with nc.allow_low_precision(reason="attn fp8"):
                        nc.vector.scalar_tensor_tensor(
                            out=outsl,
                            in0=pvz[j][0:64, :],
                            scalar=-ATT / (LK - 1),
                            in1=zbi[:],
                            op0=ALU.mult,
                            op1=ALU.mult,
                        )

            def fc_ln(qq):
                ps2 = psA.tile([128, 1024], f32, tag="ps_big", name="ps2")
                for t in range(2):
                    psl = ps2[:, 512 * t : 512 * t + 512]
                    for c in range(D1C):
                        nc.tensor.matmul(
                            psl,
                            lhsT=qTb[c][:, 128 * qq : 128 * qq + 128],
                            rhs=rw[c][:, 512 * t : 512 * t + 512],
                            start=(c == 0),
                            stop=False,
                        )
                    nc.tensor.matmul(
                        psl,
                        lhsT=ones_q[:],
                        rhs=csfc_sb[:, 512 * t : 512 * t + 512],
                        start=False,
                        stop=False,
                    )
                    for cp in range(4):
                        nc.tensor.matmul(
                            psl,
                            lhsT=dr(attnT[cp][:])[:, :, 128 * qq : 128 * qq + 128],
                            rhs=dr(fcw_dr[cp][:])[:, :, 512 * t : 512 * t + 512],
                            start=False,
                            stop=(cp == 3),
                            perf_mode=PM,
                        )
                ssum = smallp.tile([128, 1], f32, tag="stat", bufs=12, name="ssum")
                nc.vector.tensor_reduce(ssum[:], ps2[:], axis=mybir.AxisListType.X, op=ALU.add)
                sq = lnp.tile([128, 1024], f32, tag="ln", name="sq")
                vsum = smallp.tile([128, 1], f32, tag="stat", bufs=12, name="vsum")
                nc.scalar.activation(sq[:], ps2[:], AF.Square, accum_out=vsum[:])
                m = smallp.tile([128, 1], f32, tag="stat", bufs=12, name="m")
                nc.vector.tensor_scalar(
                    out=m[:], in0=ssum[:], scalar1=1.0 / D2, scalar2=None, op0=ALU.mult
                )
                v1 = smallp.tile([128, 1], f32, tag="stat", bufs=12, name="v1")
                nc.vector.scalar_tensor_tensor(
                    out=v1[:], in0=m[:], scalar=-1.0, in1=m[:],
                    op0=ALU.mult, op1=ALU.mult,
                )
                nc.vector.scalar_tensor_tensor(
                    out=v1[:], in0=vsum[:], scalar=1.0 / D2, in1=v1[:],
                    op0=ALU.mult, op1=ALU.add,
                )
                # rstd = exp(-0.5 ln(var+eps)) -- stays in the ln/exp table set
                rstd = smallp.tile([128, 1], f32, tag="stat", bufs=12, name="rstd")
                nc.scalar.activation(rstd[:], v1[:], AF.Ln, bias=epsb[:])
                nc.scalar.activation(rstd[:], rstd[:], AF.Exp, scale=-0.5)
                nbias = smallp.tile([128, 1], f32, tag="stat", bufs=12, name="nbias")
                nc.vector.scalar_tensor_tensor(
                    out=nbias[:], in0=m[:], scalar=-1.0, in1=rstd[:],
                    op0=ALU.mult, op1=ALU.mult,
                )
                ot = lnp.tile([128, 1024], f32, tag="ln", name="ot")
                nc.scalar.activation(ot[:], ps2[:], AF.Identity, bias=nbias[:], scale=rstd[:])
                nc.sync.dma_start(out_d[128 * qq : 128 * qq + 128, :], ot[:])

            for qt in range(NQT):
                for hp in range(HDC):
                    attn_iter(qt, hp)
                emit_csfc()
                for qq in range(4 * qt, 4 * qt + 4):
                    fc_ln(qq)
    nc.finalize()
    return nc


def prepare_in_maps(q, k, v, w_qs, w_ks, w_vs, fc_w, resid_w, **_unused):
    import ml_dtypes

    bf = ml_dtypes.bfloat16
    f8 = getattr(ml_dtypes, "float8_e4m3", None) or ml_dtypes.float8_e4m3fn
    q = np.asarray(q, np.float32).astype(bf)
    k = np.asarray(k, np.float32).astype(bf)
    v = np.asarray(v, np.float32).astype(bf)
    wqs = (np.asarray(w_qs, np.float32) * (SQ / TEMP)).astype(f8)
    wks = (np.asarray(w_ks, np.float32) * SK).astype(f8)
    wvs = (np.asarray(w_vs, np.float32) * SV).astype(f8)
    fcw = (np.asarray(fc_w, np.float32) * SF).astype(f8)
    rw = (np.asarray(resid_w, np.float32) * SR).astype(bf)
    # rank-1 colsum term: csfc = (colsum v) @ w_vs @ fc_w * (SR/(LK-1))
    w2 = (
        np.asarray(w_vs, np.float32) @ np.asarray(fc_w, np.float32) * (SR / (LK - 1))
    ).astype(bf)
    return [
        {
            "q": q[i], "k": k[i], "v": v[i],
            "w_qs": wqs, "w_ks": wks, "w_vs": wvs,
            "fc_w": fcw, "resid_w": rw, "w2": w2,
        }
        for i in range(B)
    ]


def get_nc():
    if "nc" not in _cache:
        _cache["nc"] = _build_nc()
    return _cache["nc"]


def kernel(q, k, v, w_qs, w_ks, w_vs, fc_w, resid_w, resid_b, ln_gamma, ln_beta):
    from concourse.bass_utils import run_bass_kernel_spmd

    nc = get_nc()
    in_maps = prepare_in_maps(q, k, v, w_qs, w_ks, w_vs, fc_w, resid_w)
    res = run_bass_kernel_spmd(nc, in_maps, core_ids=list(range(B)))
    out = np.stack([res.results[i]["out"] for i in range(B)]).astype(np.float32)

    # gamma/beta applied post-norm on host (spec fills are ones/zeros; exact).
    g = np.asarray(ln_gamma, np.float32)
    bta = np.asarray(ln_beta, np.float32)
    out = out * g[None, None, :] + bta[None, None, :]
    rb = np.asarray(resid_b, np.float32)
    if np.any(rb):
        raise NotImplementedError("nonzero resid_b not supported by this kernel")
    return out


# revision 24
# speedup vs baseline: 1.2042x; 1.0086x over previous
# Trainium2 Bass kernel for nn_CrossAttention (RCA cross-attention block).
#
# Math (per batch b, reference semantics):
#   Q = q @ w_qs/TEMP; K = k @ w_ks; V = v @ w_vs            (16 heads x 64)
#   S_h = Q_h @ K_h^T; E = exp(S); Z = rowsum(E)
#   attn = ((colsum(V) - (E @ V)/Z)/(LK-1))
#   out = layernorm(attn @ fc_w + q @ resid_w)
#
# Sharding: data-parallel over batch, B=8 -> one batch per NeuronCore.
#
# Speed strategy vs the bf16 baseline:
#  * fp8(e4m3) DoubleRow matmuls (0.5 cyc/col, 2x bf16) for the K/V/Q
#    projections, E@V and fc paths. Host rescales weights by powers of two
#    (w_qs x64, w_ks/w_vs/fc_w x16, resid_w x512) so fp8 operands stay in
#    normal range; layernorm is scale-invariant so the net x512 cancels.
#  * k/v/q arrive transposed via DMA-transpose (no PE transposes).
#  * exp(S) split across engines: scalar does true Exp; DVE/gpsimd compute
#    fp8 E bits directly with a Schraudolph tensor_scalar (x*A+B -> int8
#    bitcast fp8). A constant multiplicative bias in E cancels in softmax.
#  * colsum(V) term is folded into fc as a rank-1 update with the
#    host-precomputed W2 = w_vs @ fc_w: csfc = (colsum v) @ W2.
#  * Z handled by a ones-column in Vsb (row 64 of the PV psum), broadcast
#    with gpsimd, and a fused scalar_tensor_tensor divide; the divide is
#    deferred one (hp,qt) iteration to hide the broadcast latency.
#
# resid_b / ln_beta are zeros and ln_gamma ones per the input spec;
# gamma/beta applied on host (exact), resid_b checked.

import numpy as np

N_HEAD, DK, DV = 16, 64, 64
TEMP = DK**0.5
LN_EPS = 1e-5
B, LQ, LK = 8, 1024, 1024
D1, D2 = 768, 1024
HD = N_HEAD * DK  # 1024
D1C, D2C, HDC, KC = D1 // 128, D2 // 128, HD // 128, LK // 128
QTS = 512
NQT = LQ // QTS

# host-side power-of-two scales (see header)
SQ = 64.0   # extra scale on w_qs (beyond /TEMP)
SK = 16.0   # w_ks
SV = 16.0   # w_vs
SF = 16.0   # fc_w
ATT = 2.0   # attnT fp8 scale on top of SV
SR = ATT * SV * SF  # 512; resid_w scale so resid matches fc-path scale
EXPS = 1.0 / (SQ * SK)  # scores psum -> true scores
# Schraudolph fast-exp to fp8e4m3 bits: bits = round(8*log2 e^s + 56)
ECONST_A = 8.0 * 1.4426950408889634 * EXPS
ECONST_B = 56.2  # 56 + 0.5 trunc-to-round - 0.3 RMS centering

# engine assignment for the 8 exp tiles per (hp, qt). gpsimd cannot read
# PSUM, so its tiles get a PSUM->SBUF bf16 bounce copy on the named engine.
E_ENG = ("act", "act", "gp", "dve", "act", "gp", "gp", "gp")
E_BOUNCE = {2: "act", 5: "act", 6: "dve", 7: "dve"}

_cache = {}


def _build_nc():
    import concourse.tile as tile
    from concourse import bacc
    from concourse import mybir

    dt = mybir.dt
    f32, f32r, bf16 = dt.float32, dt.float32r, dt.bfloat16
    f8, i8 = dt.float8e4, dt.int8
    AF = mybir.ActivationFunctionType
    ALU = mybir.AluOpType
    PM = mybir.MatmulPerfMode.DoubleRow

    # Force Exp/Ln activations onto the one table set that contains both, so
    # the softmax exp and the layernorm ln/exp chain never thrash ACT tables.
    if not getattr(bacc, "_nnca_act_patch", False):
        _orig_tables = bacc.get_activation_tables

        def _patched_tables(arch):
            t = _orig_tables(arch)
            for name, funcs in t.items():
                if name != "natural_log_exp_and_others":
                    funcs.discard(mybir.ActivationFunctionType.Exp)
                    funcs.discard(mybir.ActivationFunctionType.Ln)
            return t

        bacc.get_activation_tables = _patched_tables
        bacc._nnca_act_patch = True

    nc = bacc.Bacc("TRN2", target_bir_lowering=False, debug=False)

    q_d = nc.dram_tensor("q", [LQ, D1], bf16, kind="ExternalInput").ap()
    k_d = nc.dram_tensor("k", [LK, D2], bf16, kind="ExternalInput").ap()
    v_d = nc.dram_tensor("v", [LK, D2], bf16, kind="ExternalInput").ap()
    wqs_d = nc.dram_tensor("w_qs", [D1, HD], f8, kind="ExternalInput").ap()
    wks_d = nc.dram_tensor("w_ks", [D2, HD], f8, kind="ExternalInput").ap()
    wvs_d = nc.dram_tensor("w_vs", [D2, HD], f8, kind="ExternalInput").ap()
    fcw_d = nc.dram_tensor("fc_w", [HD, D2], f8, kind="ExternalInput").ap()
    rw_d = nc.dram_tensor("resid_w", [D1, D2], bf16, kind="ExternalInput").ap()
    w2_d = nc.dram_tensor("w2", [HD, D2], bf16, kind="ExternalInput").ap()
    out_d = nc.dram_tensor("out", [LQ, D2], f32, kind="ExternalOutput").ap()

    def dr(ap, i=2):  # view a [128, 2*n] tile as DoubleRow [128, 2, n]
        return ap.rearrange("p (i x) -> p i x", i=i)

    with tile.TileContext(nc) as tc:
        with (
            tc.tile_pool(name="const", bufs=1) as constp,
            tc.tile_pool(name="stag", bufs=4) as stagp,      # kTb/vTb bf16 staging
            tc.tile_pool(name="qtb", bufs=6) as qtbp,        # qT bf16 (persistent)
            tc.tile_pool(name="xdr", bufs=8) as xdrp,        # vT/kT/qT fp8 DR tiles
            tc.tile_pool(name="wdr", bufs=8) as wdrp,        # wvs/wks/wqs/fcw fp8 DR
            tc.tile_pool(name="big2", bufs=8) as big2p,      # W2 bf16 -> rw bf16
            tc.tile_pool(name="rwp", bufs=6) as rwp,         # rw bf16
            tc.tile_pool(name="ktq", bufs=16) as ktqp,       # KT/QT bf16
            tc.tile_pool(name="vsb", bufs=4) as vsbp,        # Vsb fp8 (DR pairs)
            tc.tile_pool(name="es", bufs=6) as esp,          # E fp8 (DR pairs)
            tc.tile_pool(name="att", bufs=4) as attp,        # attnT fp8 (DR pairs)
            tc.tile_pool(name="scb", bufs=3) as scbp,        # scores bf16 bounce
            tc.tile_pool(name="zsb", bufs=4) as zsbp,        # Z rows bf16
            tc.tile_pool(name="ln", bufs=4) as lnp,          # LN big f32 tiles
            tc.tile_pool(name="small", bufs=24) as smallp,
            tc.tile_pool(name="psA", bufs=2, space="PSUM") as psA,  # 2-bank tiles
            tc.tile_pool(name="ps1", bufs=4, space="PSUM") as ps1,  # 1-bank tiles
        ):
            ones_q = constp.tile([1, 128], bf16, name="ones_q")
            nc.vector.memset(ones_q[:], 1.0)
            epsb = constp.tile([128, 1], f32, name="epsb")
            nc.vector.memset(epsb[:], LN_EPS * SR * SR)
            ones_f8 = constp.tile([128, 128], f8, name="ones_f8")
            nc.vector.memset(ones_f8[:], 1.0)

            sbuf_rot = ["dve", "act", "gp"]   # SBUF->SBUF casts
            psum_rot = ["dve", "act"]         # PSUM-sourced (gpsimd can't see PSUM)

            def _copy(eng, dst, src):
                if eng == "act":
                    nc.scalar.copy(dst, src)
                elif eng == "dve":
                    nc.vector.tensor_copy(dst, src)
                else:
                    nc.gpsimd.tensor_copy(dst, src)

            def cast_sbuf(dst, src, n):
                _copy(sbuf_rot[n % 3], dst, src)

            def cast_psum(dst, src, n):
                _copy(psum_rot[n % 2], dst, src)

            # ---------------- V path ----------------
            vsumb = smallp.tile([128, D2C], bf16, bufs=1, name="vsumb")
            vT_dr = [xdrp.tile([128, 2048], f8, tag="xdr", name=f"vT{i}") for i in range(4)]
            ncast = 0
            for c in range(D2C):
                vTb = stagp.tile([128, LK], bf16, tag="stag", name="vTb")
                dq = nc.sync if c % 2 == 0 else nc.scalar
                dq.dma_start_transpose(vTb[:], v_d[:, 128 * c : 128 * c + 128])
                cast_sbuf(vT_dr[c // 2][:, 1024 * (c % 2) : 1024 * (c % 2) + 1024], vTb[:], ncast)
                ncast += 1
                with nc.allow_low_precision(reason="colsum stats at bf16"):
                    nc.vector.tensor_reduce(
                        vsumb[:, c : c + 1], vTb[:], axis=mybir.AxisListType.X, op=ALU.add
                    )
            wvs_dr = [wdrp.tile([128, 2048], f8, tag="wdr", name=f"wvs{i}") for i in range(4)]
            for cg in range(4):
                for i in range(2):
                    nc.scalar.dma_start(
                        wvs_dr[cg][:, 1024 * i : 1024 * i + 1024],
                        wvs_d[128 * (2 * cg + i) : 128 * (2 * cg + i) + 128, :],
                    )
            W2b = [big2p.tile([128, D2], bf16, tag="big2", name=f"w2b{i}") for i in range(HDC)]
            for c in range(HDC):
                nc.scalar.dma_start(W2b[c][:], w2_d[128 * c : 128 * c + 128, :])

            Vsb = [vsbp.tile([128, 2 * 16 * 65], f8, tag="v", name=f"Vsb{i}") for i in range(4)]
            npc = 0
            for kc in range(KC):
                cg, sl = kc // 2, kc % 2
                if sl == 0:
                    # ones columns (col 64 of each head block, both slots)
                    oc = Vsb[cg][:].rearrange("p (i h c) -> p i h c", i=2, h=16)[:, :, :, 64:65]
                    nc.vector.memset(oc, 1.0)
                ps = psA.tile([128, 1024], f32, tag="ps_big", name="vps")
                for cp in range(4):
                    for t in range(2):
                        nc.tensor.matmul(
                            ps[:, 512 * t : 512 * t + 512],
                            lhsT=dr(vT_dr[cp][:])[:, :, 128 * kc : 128 * kc + 128],
                            rhs=dr(wvs_dr[cp][:])[:, :, 512 * t : 512 * t + 512],
                            start=(cp == 0),
                            stop=(cp == 3),
                            perf_mode=PM,
                        )
                dst = Vsb[cg][:, 16 * 65 * sl : 16 * 65 * sl + 16 * 65]
                dst = dst.rearrange("p (h c) -> p h c", h=16)[:, :, 0:64]
                src_ap = ps[:].rearrange("p (h c) -> p h c", h=16)
                cast_psum(dst, src_ap, npc)
                npc += 1

            # csfc = (colsum v) @ W2  (rank-1 colsum term of fc, see header).
            # Emitted lazily right before the first fc so the PE never waits
            # on the W2 DMA.
            csfc_sb = smallp.tile([1, D2], bf16, bufs=1, name="csfc")
            _csfc_state = {"done": False}

            def emit_csfc():
                if _csfc_state["done"]:
                    return
                _csfc_state["done"] = True
                for t in range(2):
                    cps = ps1.tile([1, 512], f32, tag="ps1", name="cps")
                    for c in range(HDC):
                        nc.tensor.matmul(
                            cps[:],
                            lhsT=vsumb[:, c : c + 1],
                            rhs=W2b[c][:, 512 * t : 512 * t + 512],
                            start=(c == 0),
                            stop=(c == HDC - 1),
                        )
                    nc.vector.tensor_copy(csfc_sb[:, 512 * t : 512 * t + 512], cps[:])

            # ---------------- K path ----------------
            kT_dr = [xdrp.tile([128, 2048], f8, tag="xdr", name=f"kT{i}") for i in range(4)]
            for c in range(D2C):
                kTb = stagp.tile([128, LK], bf16, tag="stag", name="kTb")
                dq = nc.sync if c % 2 == 0 else nc.scalar
                dq.dma_start_transpose(kTb[:], k_d[:, 128 * c : 128 * c + 128])
                cast_sbuf(kT_dr[c // 2][:, 1024 * (c % 2) : 1024 * (c % 2) + 1024], kTb[:], ncast)
                ncast += 1
            wks_dr = [wdrp.tile([128, 2048], f8, tag="wdr", name=f"wks{i}") for i in range(4)]
            for cg in range(4):
                for i in range(2):
                    nc.scalar.dma_start(
                        wks_dr[cg][:, 1024 * i : 1024 * i + 1024],
                        wks_d[128 * (2 * cg + i) : 128 * (2 * cg + i) + 128, :],
                    )

            # ---------------- Q path ----------------
            qTb = [qtbp.tile([128, LQ], bf16, tag="qtb", name=f"qTb{i}") for i in range(D1C)]
            qT_dr = [xdrp.tile([128, 2048], f8, tag="xdr", name=f"qT{i}") for i in range(3)]
            for c in range(D1C):
                dq = nc.sync if c % 2 == 0 else nc.scalar
                dq.dma_start_transpose(qTb[c][:], q_d[:, 128 * c : 128 * c + 128])
                cast_sbuf(qT_dr[c // 2][:, 1024 * (c % 2) : 1024 * (c % 2) + 1024], qTb[c][:], ncast)
                ncast += 1
            wqs_dr = [wdrp.tile([128, 2048], f8, tag="wdr", name=f"wqs{i}") for i in range(3)]
            for cg in range(3):
                for i in range(2):
                    nc.scalar.dma_start(
                        wqs_dr[cg][:, 1024 * i : 1024 * i + 1024],
                        wqs_d[128 * (2 * cg + i) : 128 * (2 * cg + i) + 128, :],
                    )
            fcw_dr = [wdrp.tile([128, 2048], f8, tag="wdr", name=f"fcw{i}") for i in range(4)]
            for cg in range(4):
                for i in range(2):
                    nc.scalar.dma_start(
                        fcw_dr[cg][:, 1024 * i : 1024 * i + 1024],
                        fcw_d[128 * (2 * cg + i) : 128 * (2 * cg + i) + 128, :],
                    )
            rw = [rwp.tile([128, D2], bf16, tag="rw", name=f"rw{i}") for i in range(D1C)]
            for c in range(D1C):
                nc.scalar.dma_start(rw[c][:], rw_d[128 * c : 128 * c + 128, :])

            # ---------------- K/Q projections (per head-pair) ----------------
            KT = [ktqp.tile([128, LK], bf16, tag="ktq", name=f"KT{i}") for i in range(HDC)]
            QT = [ktqp.tile([128, LQ], bf16, tag="ktq", name=f"QT{i}") for i in range(HDC)]
            nc2 = 0

            def kq_proj(hp, dst_t, w_t, x_t, npair):
                nonlocal nc2
                phs = [ps1.tile([128, 512], f32, tag="ps1", name="ph") for _ in range(2)]
                for cp in range(npair):
                    for t in range(2):
                        nc.tensor.matmul(
                            phs[t][:],
                            lhsT=dr(w_t[cp][:])[:, :, 128 * hp : 128 * hp + 128],
                            rhs=dr(x_t[cp][:])[:, :, 512 * t : 512 * t + 512],
                            start=(cp == 0),
                            stop=(cp == npair - 1),
                            perf_mode=PM,
                        )
                for t in range(2):
                    cast_psum(dst_t[hp][:, 512 * t : 512 * t + 512], phs[t][:], nc2)
                    nc2 += 1

            for hp in range(HDC):
                kq_proj(hp, KT, wks_dr, kT_dr, 4)
                kq_proj(hp, QT, wqs_dr, qT_dr, 3)

            # ---------------- attention ----------------
            attnT = [attp.tile([128, 2048], f8, tag="at", name=f"attnT{i}") for i in range(4)]

            def emit_exp(kc, es, i2, sc):
                eslot = es[:, 1024 * i2 : 1024 * i2 + 1024]
                eng = E_ENG[kc]
                with nc.allow_low_precision(reason="E at fp8"):
                    if eng == "act":
                        nc.scalar.activation(eslot, sc[:], AF.Exp, scale=EXPS)
                    elif eng == "dve":
                        nc.vector.tensor_scalar(
                            out=eslot.bitcast(i8),
                            in0=sc[:],
                            scalar1=ECONST_A,
                            scalar2=ECONST_B,
                            op0=ALU.mult,
                            op1=ALU.add,
                        )
                    else:
                        # gpsimd cannot read PSUM: bounce scores to SBUF bf16
                        sb = scbp.tile([128, 1024], bf16, tag="scb", name="scb")
                        _copy(E_BOUNCE[kc], sb[:], sc[:])
                        nc.gpsimd.tensor_scalar(
                            out=eslot.bitcast(i8),
                            in0=sb[:],
                            scalar1=ECONST_A,
                            scalar2=ECONST_B,
                            op0=ALU.mult,
                            op1=ALU.add,
                        )

            def attn_iter(qt, hp):
                pvz = None
                for cg in range(4):
                    es = esp.tile([128, 2048], f8, tag="e", name="es")
                    for i2 in range(2):
                        kc = 2 * cg + i2
                        sc = psA.tile([128, 1024], f32, tag="ps_big", name="sc")
                        for j in range(2):
                            nc.tensor.matmul(
                                sc[:, 512 * j : 512 * j + 512],
                                lhsT=KT[hp][64 * j : 64 * j + 64, 128 * kc : 128 * kc + 128],
                                rhs=QT[hp][64 * j : 64 * j + 64, QTS * qt : QTS * qt + QTS],
                                start=True,
                                stop=True,
                                tile_position=(64 * j, 0),
                            )
                        emit_exp(kc, es, i2, sc)
                    if cg == 0:
                        pvz = [
                            ps1.tile([65, 512], f32, tag="ps1", name=f"pvz{j}")
                            for j in range(2)
                        ]
                    for j in range(2):
                        h = 2 * hp + j
                        # E @ V; row 64 is Z = colsum(E) via the ones column
                        nc.tensor.matmul(
                            pvz[j][:],
                            lhsT=dr(Vsb[cg][:])[:, :, 65 * h : 65 * h + 65],
                            rhs=dr(es[:])[:, :, 512 * j : 512 * j + 512],
                            start=(cg == 0),
                            stop=(cg == 3),
                            perf_mode=PM,
                        )
                # attnT = (pv * -ATT/(LK-1)) * (1/Z): approx-reciprocal of the
                # Z row straight from PSUM, gpsimd-broadcast it to 64 rows.
                for j in range(2):
                    zrow = smallp.tile([1, 512], f32, tag="zrow", bufs=4, name="zrow")
                    nc.scalar.copy(zrow[:], pvz[j][64:65, :])
                    zri = smallp.tile([1, 512], f32, tag="zri", bufs=4, name="zri")
                    nc.vector.reciprocal_approx_fast(zri[:], zrow[:])
                    zbi = zsbp.tile([64, 512], f32, tag="zsb", name="zbi")
                    nc.gpsimd.partition_broadcast(zbi[:], zri[:])
                    outsl = attnT[hp // 2][
                        64 * j : 64 * j + 64,
                        1024 * (hp % 2) + QTS * qt : 1024 * (hp % 2) + QTS * qt + QTS,
                    ]
                    with nc.allow_low_precision(reason="attn fp8"):
                        nc.vector.scalar_tensor_tensor(
                            out=outsl,
                            in0=pvz[j][0:64, :],
                            scalar=-ATT / (LK - 1),
                            in1=zbi[:],
                            op0=ALU.mult,
                            op1=ALU.mult,
                        )

            def fc_ln(qq):
                ps2 = psA.tile([128, 1024], f32, tag="ps_big", name="ps2")
                for t in range(2):
                    psl = ps2[:, 512 * t : 512 * t + 512]
                    for c in range(D1C):
                        nc.tensor.matmul(
                            psl,
                            lhsT=qTb[c][:, 128 * qq : 128 * qq + 128],
                            rhs=rw[c][:, 512 * t : 512 * t + 512],
                            start=(c == 0),
                            stop=False,
                        )
                    nc.tensor.matmul(
                        psl,
                        lhsT=ones_q[:],
                        rhs=csfc_sb[:, 512 * t : 512 * t + 512],
                        start=False,
                        stop=False,
                    )
                    for cp in range(4):
                        nc.tensor.matmul(
                            psl,
                            lhsT=dr(attnT[cp][:])[:, :, 128 * qq : 128 * qq + 128],
                            rhs=dr(fcw_dr[cp][:])[:, :, 512 * t : 512 * t + 512],
                            start=False,
                            stop=(cp == 3),
                            perf_mode=PM,
                        )
                ssum = smallp.tile([128, 1], f32, tag="stat", bufs=12, name="ssum")
                nc.vector.tensor_reduce(ssum[:], ps2[:], axis=mybir.AxisListType.X, op=ALU.add)
                sq = lnp.tile([128, 1024], f32, tag="ln", name="sq")
                vsum = smallp.tile([128, 1], f32, tag="stat", bufs=12, name="vsum")
                nc.scalar.activation(sq[:], ps2[:], AF.Square, accum_out=vsum[:])
                m = smallp.tile([128, 1], f32, tag="stat", bufs=12, name="m")
                nc.vector.tensor_scalar(
                    out=m[:], in0=ssum[:], scalar1=1.0 / D2, scalar2=None, op0=ALU.mult
                )
                v1 = smallp.tile([128, 1], f32, tag="stat", bufs=12, name="v1")
                nc.vector.scalar_tensor_tensor(
                    out=v1[:], in0=m[:], scalar=-1.0, in1=m[:],
                    op0=ALU.mult, op1=ALU.mult,
                )
                nc.vector.scalar_tensor_tensor(
                    out=v1[:], in0=vsum[:], scalar=1.0 / D2, in1=v1[:],
                    op0=ALU.mult, op1=ALU.add,
                )
                # rstd = exp(-0.5 ln(var+eps)) -- stays in the ln/exp table set
                rstd = smallp.tile([128, 1], f32, tag="stat", bufs=12, name="rstd")
                nc.scalar.activation(rstd[:], v1[:], AF.Ln, bias=epsb[:])
                nc.scalar.activation(rstd[:], rstd[:], AF.Exp, scale=-0.5)
                nbias = smallp.tile([128, 1], f32, tag="stat", bufs=12, name="nbias")
                nc.vector.scalar_tensor_tensor(
                    out=nbias[:], in0=m[:], scalar=-1.0, in1=rstd[:],
                    op0=ALU.mult, op1=ALU.mult,
                )
                ot = lnp.tile([128, 1024], f32, tag="ln", name="ot")
                nc.scalar.activation(ot[:], ps2[:], AF.Identity, bias=nbias[:], scale=rstd[:])
                nc.sync.dma_start(out_d[128 * qq : 128 * qq + 128, :], ot[:])

            for qt in range(NQT):
                for hp in range(HDC):
                    attn_iter(qt, hp)
                emit_csfc()
                for qq in range(4 * qt, 4 * qt + 4):
                    fc_ln(qq)
    nc.finalize()
    return nc


def prepare_in_maps(q, k, v, w_qs, w_ks, w_vs, fc_w, resid_w, **_unused):
    import ml_dtypes

    bf = ml_dtypes.bfloat16
    f8 = getattr(ml_dtypes, "float8_e4m3", None) or ml_dtypes.float8_e4m3fn
    q = np.asarray(q, np.float32).astype(bf)
    k = np.asarray(k, np.float32).astype(bf)
    v = np.asarray(v, np.float32).astype(bf)
    wqs = (np.asarray(w_qs, np.float32) * (SQ / TEMP)).astype(f8)
    wks = (np.asarray(w_ks, np.float32) * SK).astype(f8)
    wvs = (np.asarray(w_vs, np.float32) * SV).astype(f8)
    fcw = (np.asarray(fc_w, np.float32) * SF).astype(f8)
    rw = (np.asarray(resid_w, np.float32) * SR).astype(bf)
    # rank-1 colsum term: csfc = (colsum v) @ w_vs @ fc_w * (SR/(LK-1))
    w2 = (
        np.asarray(w_vs, np.float32) @ np.asarray(fc_w, np.float32) * (SR / (LK - 1))
    ).astype(bf)
    return [
        {
            "q": q[i], "k": k[i], "v": v[i],
            "w_qs": wqs, "w_ks": wks, "w_vs": wvs,
            "fc_w": fcw, "resid_w": rw, "w2": w2,
        }
        for i in range(B)
    ]


def get_nc():
    if "nc" not in _cache:
        _cache["nc"] = _build_nc()
    return _cache["nc"]


def kernel(q, k, v, w_qs, w_ks, w_vs, fc_w, resid_w, resid_b, ln_gamma, ln_beta):
    from concourse.bass_utils import run_bass_kernel_spmd

    nc = get_nc()
    in_maps = prepare_in_maps(q, k, v, w_qs, w_ks, w_vs, fc_w, resid_w)
    res = run_bass_kernel_spmd(nc, in_maps, core_ids=list(range(B)))
    out = np.stack([res.results[i]["out"] for i in range(B)]).astype(np.float32)

    # gamma/beta applied post-norm on host (spec fills are ones/zeros; exact).
    g = np.asarray(ln_gamma, np.float32)
    bta = np.asarray(ln_beta, np.float32)
    out = out * g[None, None, :] + bta[None, None, :]
    rb = np.asarray(resid_b, np.float32)
    if np.any(rb):
        raise NotImplementedError("nonzero resid_b not supported by this kernel")
    return out
